# revision 2
# baseline (speedup 1.0000x reference)
"""Trainium2 Bass kernel for nn_ConvPixelToCapsules — v2.

Reference:
  x (16,256,1,20,20) --conv W (256,1,9,9) stride 2--> votes (16,256,32,8,6,6)
  3 routing iterations (softmax over co, ci-reduce, squash over no,
  agreement update) -> activation (16, 32, 8, 6, 6)

Sharding: data-parallel over batch, 2 batch elements per core, 8 cores.

v2 design (vs baseline):
  - conv in bf16 with K=27 (3 shifted x copies on 27 partitions; kx = g+3q,
    3 PSUM-accumulated steps q): 432 matmuls of N=256 @1cyc/row.
  - votes stored bf16 [128 planes, (no8, hw36, co32)] per chunk: the
    route-broadcast multiply gets DVE 2x mode (route AP [[0,8],[32,36],[1,32]]).
  - ci-reduce: ones-stationary streaming matmuls (M=1, N=384), PSUM [1,384]
    tiles, evacuated by tiny DMAs from partition 0 into a compact
    [36=(b2,hwA18), 512=(hwB2,no8,co32)] preact tile; bias+squash there.
  - distances: mult (2x) + bf16 tree-add over no (2x) instead of 1x reduce.
  - act broadcast [128, 9216] via adr-DRAM bounce + 8x16-partition DMAs.
"""

import sys
import functools
import numpy as np

sys.path.insert(0, "/opt/trn_rl_repo")

import concourse.bass as bass  # noqa: E402
import concourse.tile as tile  # noqa: E402
from concourse import mybir  # noqa: E402
from concourse.bass_utils import run_bass_kernel_spmd  # noqa: E402

F32 = mybir.dt.float32
BF16 = mybir.dt.bfloat16

BS, CI, HI, WI = 16, 256, 20, 20
CO, NO, ITERS = 32, 8, 3
HOUT = WOUT = 6
POS = HOUT * WOUT            # 36
NCORES = 8
BSH = BS // NCORES           # 2
PLANES = BSH * CI            # 512
NCHUNK = PLANES // 128       # 4
CONO = CO * NO               # 256
CHW = NO * POS * CO          # 9216 free elems per chunk: (no, hw, co)
HWCO = POS * CO              # 1152

Alu = mybir.AluOpType
Act = mybir.ActivationFunctionType


def ap(t, offset, dims):
    return bass.AP(tensor=t.tensor, offset=t.offset + offset, ap=[list(d) for d in dims])


def _split_excess_waits(nc):
    """Walrus allows only ONE sync-wait on DMA and Matmult/Ldweights pseudo
    structs; splice same-engine NoOps carrying overflow waits."""
    import bass_rust

    nid = 0
    for f in nc.m.functions:
        for blk in f.blocks:
            out = []
            changed = False
            for ins in blk.instructions:
                si = ins.sync_info
                if si is not None and len(si.on_wait) > 1:
                    extra = list(si.on_wait)[:-1]
                    keep = list(si.on_wait)[-1:]
                    for w in extra:
                        nop = bass_rust.InstNoOp(name=f"I-waitnop-{nid}")
                        nid += 1
                        nop.engine = ins.engine
                        nop.sync_info = bass_rust.SyncInfo(on_wait=[w], on_update=[])
                        out.append(nop)
                    ins.sync_info = bass_rust.SyncInfo(
                        on_wait=keep, on_update=list(si.on_update))
                    changed = True
                out.append(ins)
            if changed:
                blk.instructions = out


def build_program():
    nc = bass.Bass("TRN2", target_bir_lowering=False, debug=False)
    x_d = nc.dram_tensor("x", [PLANES, HI * WI], BF16, kind="ExternalInput").ap()
    w_d = nc.dram_tensor("w", [27, 3 * CONO], BF16, kind="ExternalInput").ap()
    b_d = nc.dram_tensor("b", [512], F32, kind="ExternalInput").ap()
    # compact output [36=(b2,hwA18), 512=(hwB2,no8,co32)]; host reorders.
    out_d = nc.dram_tensor("out", [POS, 512], F32, kind="ExternalOutput").ap()

    with tile.TileContext(nc) as tc:
        _emit(tc, nc, x_d, w_d, b_d, out_d)
    _split_excess_waits(nc)
    return nc


def _emit(tc, nc, x_d, w_d, b_d, out_d):
    import contextlib

    with contextlib.ExitStack() as ctx:
        persist = ctx.enter_context(tc.tile_pool(name="persist", bufs=1))
        dram = ctx.enter_context(tc.tile_pool(name="dram", bufs=2, space="DRAM"))
        rt_ps = ctx.enter_context(tc.tile_pool(name="rt_ps", bufs=3, space="PSUM"))
        pools = {}
        pools["abcp"] = ctx.enter_context(tc.tile_pool(name="abcp", bufs=2))
        pools["abcph"] = ctx.enter_context(tc.tile_pool(name="abcph", bufs=1))
        small = ctx.enter_context(tc.tile_pool(name="small", bufs=1))
        cpool = ctx.enter_context(tc.tile_pool(name="cpool", bufs=1))

        votes = [persist.tile([128, CHW], BF16, name=f"votes{c}") for c in range(NCHUNK)]
        logits = [persist.tile([128, HWCO], F32, name=f"logits{c}") for c in range(NCHUNK)]
        biasC = persist.tile([18, 512], F32, name="biasC")
        ones_bf = nc.const_aps.tensor(1.0, (128, 1), BF16)
        one27 = nc.const_aps.tensor(1.0, (27, 1), BF16)

        nc.sync.dma_start(out=biasC[:], in_=ap(b_d, 0, [[0, 18], [1, 512]]))

        evc = [0]

        def emit_B(t, b, red_src, dve_split=False, pool_dma=False):
            """ci-reduce for batch b: 24 ones-matmuls N=384 -> PSUM [1,384]
            -> copies into 1-partition sg strip (cp-order) -> one scatter DMA
            -> bias+squash -> actC + adr/abc bcast (or final output)."""
            last = t == ITERS - 1
            sg = [small.tile([1, CHW // 3], BF16, name="sg", tag=f"sg{i}")
                  for i in range(3)]
            cp = cpool.tile([18, 512], BF16, name="cp", tag=f"cp{b}")
            dma_q = nc.gpsimd.dma_start if pool_dma else nc.scalar.dma_start
            # j-order: all no for hwt=0, then hwt=1, hwt=2 -> early scatters
            for jj in range(24):
                hwt, no = jj // 8, jj % 8
                pp = rt_ps.tile([1, 384], F32, name="pp", tag="pp")
                j = no * 3 + hwt
                for k, c in enumerate((2 * b, 2 * b + 1)):
                    rhs = ap(red_src[c][:], j * 384, [[CHW, 128], [1, 384]])
                    nc.tensor.matmul(pp[:], ones_bf, rhs,
                                     start=(k == 0), stop=(k == 1))
                sgdst = ap(sg[hwt][:], no * 64,
                           [[CHW // 3, 1], [512, 6], [CO, 2], [1, CO]])
                if dve_split and jj % 2 == 0:
                    nc.vector.tensor_copy(out=sgdst, in_=pp[:])
                else:
                    nc.scalar.copy(out=sgdst, in_=pp[:])
                if no == 7:
                    dma_q(out=ap(cp[:], (b * 0 + 6 * hwt) * 512,
                                 [[512, 6], [1, 512]]),
                          in_=sg[hwt][:])

            pbt = F32 if last else BF16
            pb = small.tile([18, 512], pbt, name="pb", tag=f"pb{b}")
            nc.vector.scalar_tensor_tensor(
                pb[:], cp[:], (1.0 / CO) if t == 0 else 1.0, biasC[:],
                Alu.mult, Alu.add)
            sq = small.tile([18, 512], BF16, name="sq", tag="sq")
            nc.scalar.activation(out=sq[:], in_=pb[:], func=Act.Square, scale=1.0)
            t1 = small.tile([18, 256], BF16, name="t1", tag="t1")
            nc.vector.tensor_tensor(
                t1[:],
                ap(sq[:], 0, [[512, 18], [64, 4], [1, 64]]),
                ap(sq[:], 256, [[512, 18], [64, 4], [1, 64]]), Alu.add)
            t2 = small.tile([18, 128], BF16, name="t2", tag="t2")
            nc.vector.tensor_tensor(
                t2[:],
                ap(t1[:], 0, [[256, 18], [64, 2], [1, 64]]),
                ap(t1[:], 128, [[256, 18], [64, 2], [1, 64]]), Alu.add)
            s2 = small.tile([18, 64], F32, name="s2", tag=f"s2{b}")
            nc.vector.tensor_tensor(
                s2[:],
                ap(t2[:], 0, [[128, 18], [1, 64]]),
                ap(t2[:], 64, [[128, 18], [1, 64]]), Alu.add)
            nrm = small.tile([18, 64], F32, name="nrm", tag=f"nrm{b}")
            nc.scalar.activation(out=nrm[:], in_=s2[:], func=Act.Sqrt, scale=1.0)
            nc.vector.tensor_scalar_add(s2[:], s2[:], 1.0)
            nc.vector.reciprocal(out=s2[:], in_=s2[:])
            fac = small.tile([18, 64], pbt, name="fac", tag=f"fac{b}")
            nc.vector.tensor_tensor(fac[:], nrm[:], s2[:], Alu.mult)
            fac_b = ap(fac[:], 0, [[64, 18], [0, NO], [1, 64]])
            if last:
                # in-place: pb *= fac (write lags read in-stream)
                nc.vector.tensor_tensor(pb[:], pb[:], fac_b, Alu.mult)
                nc.sync.dma_start(
                    out=ap(out_d, b * 18 * 512, [[512, 18], [1, 512]]),
                    in_=pb[:])
                return None
            actC = cpool.tile([18, 512], BF16, name="actC", tag=f"aC{b}")
            nc.vector.tensor_tensor(actC[:], pb[:], fac_b, Alu.mult)
            adr = [dram.tile([1, CHW // 2], BF16, name="adr", tag=f"adr{b}{i}")
                   for i in range(2)]
            for i in range(2):
                dma_q(out=ap(adr[i][:], 0, [[64, 18], [HWCO, 4], [1, 64]]),
                      in_=ap(actC[:], i * 4 * 64, [[512, 18], [64, 4], [1, 64]]))
            return adr

        def emit_abc(adr, all_pool=False, pool_=None):
            H = CHW // 2
            if pool_ is not None:
                pls = [pool_, pool_]
            else:
                pls = [pools["abcp"], pools["abcph"]]
            abc = [pls[i].tile([128, H], BF16, name="abc", tag=f"abc{i}")
                   for i in range(2)]
            for i in range(2):
                for g in range(8):
                    if all_pool or g % 2 == 1:
                        dma = nc.gpsimd.dma_start
                    else:
                        dma = nc.scalar.dma_start
                    dma(out=ap(abc[i][:], g * 16 * H, [[H, 16], [1, H]]),
                        in_=ap(adr[i][:], 0, [[0, 16], [1, H]]))
            return abc

        def emit_CD(t, b, abc):
            """distances: mc = votes*abc, bf16 tree-add over no, logits +="""
            H = CHW // 2
            for c in (2 * b, 2 * b + 1):
                # products into a dedicated tree buffer; tree fully in-place
                mx = pools["work"].tile([128, CHW], BF16, name="mx", tag="mx")
                for i in range(2):
                    nc.vector.tensor_tensor(
                        ap(mx[:], i * H, [[CHW, 128], [1, H]]),
                        ap(votes[c][:], i * H, [[CHW, 128], [1, H]]),
                        abc[i][:], Alu.mult)
                for piece in (H, CHW // 4, HWCO):
                    nc.vector.tensor_tensor(
                        ap(mx[:], 0, [[CHW, 128], [1, piece]]),
                        ap(mx[:], 0, [[CHW, 128], [1, piece]]),
                        ap(mx[:], piece, [[CHW, 128], [1, piece]]), Alu.add)
                if t == 0:
                    nc.vector.tensor_copy(
                        out=logits[c][:],
                        in_=ap(mx[:], 0, [[CHW, 128], [1, HWCO]]))
                else:
                    nc.vector.tensor_tensor(
                        logits[c][:], logits[c][:],
                        ap(mx[:], 0, [[CHW, 128], [1, HWCO]]), Alu.add)

        def emit_A(t, c):
            """softmax route for chunk c + mr = votes*route (2x bcast)."""
            rexp = pools["wk2"].tile([128, HWCO], BF16, name="rexp", tag="rexp")
            nc.scalar.activation(out=rexp[:], in_=logits[c][:],
                                 func=Act.Exp, scale=1.0)
            zs = small.tile([128, POS], F32, name="zs", tag="zs")
            nc.vector.reduce_sum(
                out=zs[:], in_=ap(rexp[:], 0, [[HWCO, 128], [CO, POS], [1, CO]]),
                axis=mybir.AxisListType.X)
            rz = small.tile([128, POS], F32, name="rz", tag="rz")
            nc.vector.reciprocal(out=rz[:], in_=zs[:])
            route = rexp  # in-place: rexp *= 1/Z
            nc.vector.tensor_tensor(
                route[:], rexp[:],
                ap(rz[:], 0, [[POS, 128], [1, POS], [0, CO]]), Alu.mult)
            mr = pools["work"].tile([128, CHW], BF16, name="mr", tag=f"mr{c % 2}")
            r_b = ap(route[:], 0, [[HWCO, 128], [0, NO], [CO, POS], [1, CO]])
            nc.vector.tensor_tensor(mr[:], votes[c][:], r_b, Alu.mult)
            return mr

        # ================= CONV (t0 B-phase interleaved) =================
        abc0 = {}
        with tc.tile_pool(name="conv_in", bufs=1) as conv_in, \
             tc.tile_pool(name="conv_xr", bufs=2) as conv_xr, \
             tc.tile_pool(name="conv_dps", bufs=1, space="PSUM") as conv_dps, \
             tc.tile_pool(name="conv_ps", bufs=4, space="PSUM") as conv_ps:
            w_sb = conv_in.tile([27, 3 * CONO], BF16, name="w_sb")
            nc.sync.dma_start(out=w_sb[:], in_=ap(w_d, 0, [[3 * CONO, 27], [1, 3 * CONO]]))
            dps = conv_dps.tile([1, 1], F32, name="dps", tag="dummy")
            nc.tensor.matmul(dps[:], ap(w_sb[:], 0, [[3 * CONO, 27], [1, 1]]),
                             one27, start=True, stop=True)
            for c in range(NCHUNK):
                for h in range(2):
                    xr = conv_xr.tile([27, 128, 97], BF16, name="xr", tag="xr")
                    for g in range(3):
                        for ph in range(2):
                            src = ap(x_d,
                                     c * 128 * 400 + ph * 64 * 400 + 120 * h + g,
                                     [[20, 9], [400, 64], [1, 97]])
                            dst = ap(xr[:], (9 * g) * 12416 + ph * 64 * 97,
                                     [[12416, 9], [97, 64], [1, 97]])
                            nc.sync.dma_start(out=dst, in_=src)
                    dps = conv_dps.tile([1, 1], F32, name="dps", tag="dummy")
                    nc.tensor.matmul(dps[:], ap(xr[:], 0, [[12416, 27], [1, 1]]),
                                     one27, start=True, stop=True)
                    for oyp in range(3):
                        oy = 3 * h + oyp
                        for oxp in range(3):
                            ps = conv_ps.tile([128, 2 * CONO], F32, name="cps", tag="cps")
                            for oxi in range(2):
                                ox = 2 * oxp + oxi
                                for q in range(3):
                                    lhsT = ap(xr[:], 40 * oyp + 2 * ox + 3 * q,
                                              [[12416, 27], [97, 128]])
                                    rhs = ap(w_sb[:], q * CONO, [[3 * CONO, 27], [1, CONO]])
                                    nc.tensor.matmul(
                                        ap(ps[:], oxi * CONO, [[2 * CONO, 128], [1, CONO]]),
                                        lhsT, rhs, start=(q == 0), stop=(q == 2))
                            hw0 = oy * WOUT + 2 * oxp
                            dst = ap(votes[c][:], hw0 * CO,
                                     [[CHW, 128], [CO, 2], [HWCO, NO], [1, CO]])
                            if evc[0] % 2 == 0:
                                nc.vector.tensor_copy(out=dst, in_=ps[:])
                            else:
                                nc.scalar.copy(out=dst, in_=ps[:])
                            evc[0] += 1
                if c == 1:
                    adr00 = emit_B(0, 0, votes, dve_split=True, pool_dma=True)
                    abc0[0] = emit_abc(adr00, all_pool=True)
            adr01 = emit_B(0, 1, votes, dve_split=True, pool_dma=True)

        # ================= ROUTING (software-pipelined) =================
        pools["work"] = ctx.enter_context(tc.tile_pool(name="work", bufs=1))
        pools["wk2"] = ctx.enter_context(tc.tile_pool(name="wk2", bufs=1))
        mr = [None] * NCHUNK
        abc01 = emit_abc(adr01)
        emit_CD(0, 0, abc0[0])
        mr[0] = emit_A(1, 0)
        mr[1] = emit_A(1, 1)
        emit_CD(0, 1, abc01)
        adr10 = emit_B(1, 0, mr)
        mr[2] = emit_A(1, 2)
        mr[3] = emit_A(1, 3)
        abc10 = emit_abc(adr10)
        adr11 = emit_B(1, 1, mr)
        abc11 = emit_abc(adr11)
        emit_CD(1, 0, abc10)
        mr[0] = emit_A(2, 0)
        mr[1] = emit_A(2, 1)
        emit_CD(1, 1, abc11)
        emit_B(2, 0, mr)
        mr[2] = emit_A(2, 2)
        mr[3] = emit_A(2, 3)
        emit_B(2, 1, mr, dve_split=True)


@functools.cache
def _program():
    return build_program()


def kernel(x, W, bias, **_ignored):
    import ml_dtypes
    x = np.asarray(x, dtype=np.float32)
    W = np.asarray(W, dtype=np.float32)
    bias = np.asarray(bias, dtype=np.float32)
    nc = _program()

    # w_sb[(g,ky), (q, no, co)] = W[co*8+no, 0, ky, 3q+g]
    w = W.reshape(CO, NO, 9, 9).reshape(CO, NO, 9, 3, 3)   # [co,no,ky,q,g]
    w = np.transpose(w, (4, 2, 3, 1, 0)).reshape(27, 3 * CONO)
    w_flat = np.ascontiguousarray(w).astype(ml_dtypes.bfloat16)
    # biasC free (hwB2, no8, co32) = bias[co, no]
    bc = np.broadcast_to(bias.reshape(CO, NO).T[:, None, :], (NO, 2, CO))
    b_flat = np.ascontiguousarray(bc.reshape(512).astype(np.float32))
    in_maps = []
    for i in range(NCORES):
        xs = x[i * BSH:(i + 1) * BSH].reshape(PLANES, HI * WI)
        in_maps.append({
            "x": np.ascontiguousarray(xs).astype(ml_dtypes.bfloat16),
            "w": w_flat,
            "b": b_flat,
        })
    res = run_bass_kernel_spmd(nc, in_maps, list(range(NCORES)))
    outs = []
    for i in range(NCORES):
        o = res.results[i]["out"].reshape(BSH, 18, NO, 2, CO)
        o = np.transpose(o, (0, 4, 2, 1, 3)).reshape(BSH, CO, NO, HOUT, WOUT)
        outs.append(o)
    return np.ascontiguousarray(np.concatenate(outs, axis=0).astype(np.float32))


if __name__ == "__main__":
    xs = np.random.randn(BS, CI, 1, HI, WI).astype(np.float32)
    ws = (np.random.randn(CONO, 1, 9, 9) * 0.05).astype(np.float32)
    bs_ = (np.random.randn(CO, NO, 1, 1) * 0.01).astype(np.float32)
    y = kernel(xs, ws, bs_, quantization_bits=8, quantization_bits_routing=8)
    print(y.shape, y.dtype)
